# revision 46
# baseline (speedup 1.0000x reference)
"""Trainium2 Bass kernel for nn_AttentionBiasHead.

Per-sample attention with a post-softmax additive bias produced by an MLP whose
output Linear is huge (128 x 262144).  Strategy (8 NeuronCores):

- Data-parallel over batch: core i owns samples [4i, 4i+4).
- The bias-MLP output Linear (Wb2) is column-sharded by l1: core i computes the
  bias rows l1 in [64i, 64(i+1)) for ALL 32 samples, TRANSPOSED to l2-major
  layout, then delivers each peer's 4-sample slice directly into the peer's
  SBUF with XOR-relative remote_dma (no DRAM bounce, no ncfw collective).
- Sample groups are XOR-permuted on the host (group g of core i = samples of
  core i^g) and the attention-side l1 row order is XOR-permuted the same way
  (row block m = rows of source i^m), so every remote send/receive slot is a
  compile-time-static AP in an SPMD program.
- The attention softmax runs transposed (scores^T with l2 on partitions): the
  denominator comes from a ones-vector matmul over partitions, the reciprocal
  is broadcast back with a rank-1 matmul, and the bias lands pre-transposed, so
  no tensor-engine transposes sit on the post-exchange critical path.
"""

import numpy as np

N_CORES = 8
B, L, DIN, DQ, DS, DMLP = 32, 512, 512, 128, 256, 128
BPC = B // N_CORES          # samples per core = 4
NSH = L * L // N_CORES      # bias-shard columns per core = 32768
KT = DIN // 128             # contraction tiles for projections = 4
NC1 = L // 128              # 128-row chunks per l dim = 4
NQ = 16                     # phase-B column groups (4 l1loc rows each)
SCALE = 1.0 / float(np.sqrt(DQ))
DEBUG = False

_cache = {}


def _build():
    if "nc" in _cache:
        return _cache["nc"]

    from contextlib import ExitStack

    import concourse.mybir as mybir
    import concourse.tile as tile
    from concourse import bacc
    from concourse.bass import ts, _add_dep_helper
    from concourse.masks import make_identity

    dt = mybir.dt
    f32, f16, u8 = dt.float32, dt.float16, dt.uint8

    nc = bacc.Bacc("TRN2", target_bir_lowering=False, debug=False,
                   num_devices=N_CORES, num_swdge_queues=4)

    # ---- per-core external tensors -------------------------------------
    qT_d = nc.dram_tensor("qT", [BPC, 128, KT, L], f16, kind="ExternalInput").ap()
    kT_d = nc.dram_tensor("kT", [BPC, 128, KT, L], f16, kind="ExternalInput").ap()
    vT_d = nc.dram_tensor("vT", [BPC, 128, KT, L], f16, kind="ExternalInput").ap()
    mkT_d = nc.dram_tensor("mkT", [BPC, 128, NC1, L], u8, kind="ExternalInput").ap()
    sfT_d = nc.dram_tensor("sfT", [128, DS // 128, B], f32, kind="ExternalInput").ap()
    wqkv_d = nc.dram_tensor("wqkv", [128, KT, 3, DQ], f16, kind="ExternalInput").ap()
    bias4_d = nc.dram_tensor("bias4", [128, 4], f32, kind="ExternalInput").ap()
    Wb1_d = nc.dram_tensor("Wb1", [128, DS // 128, DMLP], f32, kind="ExternalInput").ap()
    Wb2s_d = nc.dram_tensor("Wb2s", [DMLP, NSH], f16, kind="ExternalInput").ap()
    bb2T_d = nc.dram_tensor("bb2T", [128, NC1, L], f16, kind="ExternalInput").ap()
    out_d = nc.dram_tensor("out", [BPC, L, DQ], f16, kind="ExternalOutput").ap()
    if DEBUG:
        dbgB_d = nc.dram_tensor("dbgB", [128, 8192], f16, kind="ExternalOutput").ap()
        dbgR_d = nc.dram_tensor("dbgR", [128, 8192], f16, kind="ExternalOutput").ap()
        dbgE_d = nc.dram_tensor("dbgE", [128, 512], f16, kind="ExternalOutput").ap()
        dbgRec_d = nc.dram_tensor("dbgRec", [128, 512], f16, kind="ExternalOutput").ap()

    with tile.TileContext(nc) as tc, ExitStack() as ctx:
        consts = ctx.enter_context(tc.tile_pool(name="consts", bufs=1))

        # exchange buffers: BIG = my shard (transposed, grouped by dest),
        # RECV = assembled bias^T for my own 4 samples, written by peers
        BIG = consts.tile([128, N_CORES, BPC, NC1, 64], f16, name="BIG")
        RECV = consts.tile([128, N_CORES, BPC, NC1, 64], f16, name="RECV")

        # masks early on the sync queue (small, needed by phase C exp)
        mskp = ctx.enter_context(tc.tile_pool(name="mskp", bufs=BPC))
        mtile = {}
        for s in range(BPC):
            mtile[s] = mskp.tile([128, NC1, L], u8, tag="mt", name=f"mt{s}")
            nc.sync.dma_start(mtile[s][:], mkT_d[s])

        bb2T_sb = consts.tile([128, NC1, L], f16)
        nc.gpsimd.dma_start(bb2T_sb[:], bb2T_d[:])
        wqkv_sb = consts.tile([128, KT, 3, DQ], f16)
        nc.gpsimd.dma_start(wqkv_sb[:], wqkv_d[:])

        ident16 = consts.tile([128, 128], f16)
        make_identity(nc, ident16)
        c1 = consts.tile([128, 1], f16)
        nc.vector.memset(c1, 1.0)
        onesrow = consts.tile([1, 128], f16)
        nc.vector.memset(onesrow, 1.0)

        sfT_sb = consts.tile([128, DS // 128, B], f32)
        nc.gpsimd.dma_start(sfT_sb[:], sfT_d[:])
        Wb1_sb = consts.tile([128, DS // 128, DMLP], f32)
        nc.gpsimd.dma_start(Wb1_sb[:], Wb1_d[:])
        bias4_sb = consts.tile([128, 4], f32)
        nc.gpsimd.dma_start(bias4_sb[:], bias4_d[:])

        dram = ctx.enter_context(tc.tile_pool(name="dram", bufs=1, space="DRAM"))
        a2a_in = dram.tile([N_CORES, 128, 1024], f16, name="a2a_in")
        a2a_out = dram.tile([N_CORES, 128, 1024], f16, name="a2a_out")

        # ---- phase A: H^T = relu(Wb1^T @ sf^T + bb1)  [128, 32] --------
        with tc.tile_pool(name="htps", bufs=1, space="PSUM") as htps:
            ht_ps = htps.tile([128, 512], f32, name="ht_ps")
            for kt in range(DS // 128):
                nc.tensor.matmul(ht_ps[:, :B], Wb1_sb[:, kt], sfT_sb[:, kt],
                                 start=(kt == 0), stop=(kt == DS // 128 - 1))
            HT_sb = consts.tile([128, B], f16)
            nc.scalar.activation(HT_sb[:], ht_ps[:, :B],
                                 mybir.ActivationFunctionType.Relu,
                                 bias=bias4_sb[:, 3:4], scale=1.0)

        # ---- phase B: bias shard GEMM (4-stacked) + transpose to BIG ---
        pB = ctx.enter_context(tc.tile_pool(name="pB", bufs=2, space="PSUM"))
        pT = ctx.enter_context(tc.tile_pool(name="pT", bufs=2, space="PSUM"))
        stp = ctx.enter_context(tc.tile_pool(name="stp", bufs=3))

        NG = 8
        big_writes = []
        with tc.tile_pool(name="w2", bufs=4) as w2p:
            w2ts = []
            for g in range(64 // NG):
                w2t = w2p.tile([128, NG, 512], f16, tag="w2t", name=f"w2t{g}")
                # split the 8MB Wb2 stream across two DMA queues
                eng = nc.gpsimd if g % 2 == 0 else nc.scalar
                w2d = eng.dma_start(
                    w2t[:], Wb2s_d[:, ts(g, NG * 512)].rearrange(
                        "p (n w) -> p n w", w=512))
                w2ts.append(w2t)
                w2t_last = w2d
            for q in range(NQ):
                bp = pB.tile([128, 512], f32, tag="bp", name=f"bp{q}")
                for n in range(4):
                    t = 4 * q + n
                    nc.tensor.matmul(bp[ts(n, 32), :], HT_sb[:],
                                     w2ts[t // NG][:, t % NG],
                                     start=True, stop=True,
                                     tile_position=(0, 32 * n))
                st = stp.tile([128, 512], f16, tag="st", name=f"st{q}")
                eng = nc.scalar.copy if q % 2 == 0 else nc.vector.tensor_copy
                eng(st[:], bp[:])
                tp_ = pT.tile([128, 512], f16, tag="tp", name=f"tp{q}",
                              padded_shape=[128, 1024])
                for b in range(NC1):
                    nc.tensor.transpose(tp_[:, ts(b, 128)], st[:, ts(b, 128)],
                                        ident16)
                # scatter transposed block into BIG[(p), g, u, b, qn=4q+n]
                src = tp_[:].rearrange("p (b n g u) -> p b n g u",
                                       b=4, n=4, g=8, u=4)
                dst = BIG[:, :, :, :, ts(q, 4)].rearrange(
                    "p g u b n -> p b n g u")
                eng2 = nc.vector.tensor_copy if q % 2 == 0 else nc.scalar.copy
                big_writes.append(eng2(dst, src))

        # stage all 8 dest-shards to DRAM and run the AllToAll; ncfw does
        # the cross-core rendezvous internally (no in-kernel barrier)
        for g in range(N_CORES):
            eng = (nc.sync, nc.scalar, nc.gpsimd)[g % 3]
            eng.dma_start(
                a2a_in[g], BIG[:, g].rearrange("p u b q -> p (u b q)"))
        nc.gpsimd.collective_compute(
            "AllToAll", mybir.AluOpType.bypass,
            replica_groups=[list(range(N_CORES))],
            ins=[a2a_in.opt()], outs=[a2a_out.opt()],
        )
        for i in range(N_CORES):
            eng = (nc.sync, nc.scalar)[i % 2]
            eng.dma_start(
                RECV[:, i].rearrange("p u b q -> p (u b q)"), a2a_out[i])

        # ---- input loads: qk on sync (after Wb2), v on scalar gated ----
        inp = ctx.enter_context(tc.tile_pool(name="inp", bufs=BPC))
        qTin, kTin, vTin = {}, {}, {}
        for s in range(BPC):
            qTin[s] = inp.tile([128, KT, L], f16, tag="qTin", name=f"qTin{s}")
            qd = nc.sync.dma_start(qTin[s][:], qT_d[s])
            kTin[s] = inp.tile([128, KT, L], f16, tag="kTin", name=f"kTin{s}")
            kd = nc.sync.dma_start(kTin[s][:], kT_d[s])
            if s == 0:
                for dd in (qd, kd):
                    _add_dep_helper(dd.ins, w2t_last.ins, sync=True,
                                    reason="defer qk loads behind Wb2 stream")
        for s in range(BPC):
            vTin[s] = inp.tile([128, KT, L], f16, tag="vTin", name=f"vTin{s}")
            vd = nc.scalar.dma_start(vTin[s][:], vT_d[s])
            _add_dep_helper(vd.ins, w2t_last.ins, sync=True,
                            reason="defer v loads behind Wb2 stream")

        # ---- phase C PRE: everything independent of the exchange -------
        prj = ctx.enter_context(tc.tile_pool(name="prj", bufs=2))
        vpool = ctx.enter_context(tc.tile_pool(name="vpool", bufs=BPC))
        expp = ctx.enter_context(tc.tile_pool(name="expp", bufs=6))
        srp = ctx.enter_context(tc.tile_pool(name="srp", bufs=BPC * NC1))
        smal = ctx.enter_context(tc.tile_pool(name="smal", bufs=2 * BPC))
        rpool = ctx.enter_context(tc.tile_pool(name="rpool", bufs=BPC))
        pP = ctx.enter_context(tc.tile_pool(name="pP", bufs=2, space="PSUM"))
        pSC = ctx.enter_context(tc.tile_pool(name="pSC", bufs=2, space="PSUM"))
        atp = ctx.enter_context(tc.tile_pool(name="atp", bufs=NC1 + 2))
        outp = ctx.enter_context(tc.tile_pool(name="outp", bufs=3))
        SR_t, v_t = {}, {}

        for s in range(BPC):
            # projections (contraction over din on partitions)
            q_ps = pP.tile([128, 512], f32, tag="pp", name=f"qps{s}")
            for kt in range(KT):
                nc.tensor.matmul(q_ps[:], wqkv_sb[:, kt, 0], qTin[s][:, kt],
                                 start=(kt == 0), stop=(kt == KT - 1))
            qT_sb = prj.tile([128, L], f16, tag="qT", name=f"qT{s}")
            nc.scalar.activation(qT_sb[:], q_ps[:],
                                 mybir.ActivationFunctionType.Identity,
                                 bias=bias4_sb[:, 0:1], scale=1.0)

            k_ps = pP.tile([128, 512], f32, tag="pp", name=f"kps{s}")
            for kt in range(KT):
                nc.tensor.matmul(k_ps[:], wqkv_sb[:, kt, 1], kTin[s][:, kt],
                                 start=(kt == 0), stop=(kt == KT - 1))
            kT_sb = prj.tile([128, L], f16, tag="kT", name=f"kT{s}")
            nc.scalar.activation(kT_sb[:], k_ps[:],
                                 mybir.ActivationFunctionType.Identity,
                                 bias=bias4_sb[:, 1:2], scale=1.0)

            w_ps = pP.tile([128, 512], f32, tag="pp", name=f"wps{s}")
            for kt in range(KT):
                nc.tensor.matmul(w_ps[:], wqkv_sb[:, kt, 2], vTin[s][:, kt],
                                 start=(kt == 0), stop=(kt == KT - 1))
            vT_sb = prj.tile([128, L], f16, tag="vTs", name=f"vTs{s}")
            nc.vector.tensor_scalar_add(vT_sb[:], w_ps[:], bias4_sb[:, 2:3])
            v_ps = pT.tile([128, 512], f16, tag="tp", name=f"vps{s}",
                           padded_shape=[128, 1024])
            for j in range(NC1):
                nc.tensor.transpose(v_ps[:, ts(j, 128)], vT_sb[:, ts(j, 128)],
                                    ident16)
            v_sb = vpool.tile([128, NC1, DQ], f16, tag="v", name=f"v{s}")
            nc.vector.tensor_copy(v_sb[:], v_ps[:].rearrange(
                "p (j d) -> p j d", j=NC1))
            v_t[s] = v_sb

            # transposed scores + exp, per l2-chunk
            expT = {}
            for c in range(NC1):
                sc_ps = pSC.tile([128, 512], f32, tag="sp", name=f"sc{s}_{c}")
                nc.tensor.matmul(sc_ps[:], kT_sb[:, ts(c, 128)], qT_sb[:],
                                 start=True, stop=True)
                e = expp.tile([128, L], f16, tag="exp", name=f"exp{s}_{c}")
                nc.scalar.activation(e[:], sc_ps[:],
                                     mybir.ActivationFunctionType.Exp,
                                     bias=0.0, scale=SCALE)
                nc.vector.copy_predicated(e[:], mtile[s][:, c],
                                          c1[:].to_broadcast([128, 512]))
                expT[c] = e

            # denominator over partitions (all l2) via ones-matmul
            den = pSC.tile([1, 512], f32, tag="sp", name=f"den{s}",
                           padded_shape=[128, 512])
            for c in range(NC1):
                nc.tensor.matmul(den[:], c1[:], expT[c][:],
                                 start=(c == 0), stop=(c == NC1 - 1))
            rec32 = smal.tile([1, 512], f32, tag="r32", name=f"r32_{s}")
            nc.vector.reciprocal(rec32[:], den[:])
            rec16 = smal.tile([1, 512], f16, tag="r16", name=f"r16_{s}")
            nc.vector.tensor_copy(rec16[:], rec32[:])
            # broadcast reciprocal across partitions: ones[1,128]^T x rec16
            r_ps = pB.tile([128, 512], f32, tag="bp", name=f"rps{s}")
            nc.tensor.matmul(r_ps[:], onesrow[:], rec16[:],
                             start=True, stop=True)
            R_sb = rpool.tile([128, 512], f16, tag="R", name=f"R{s}")
            nc.scalar.copy(R_sb[:], r_ps[:])

            # SR = exp^T * R + bb2^T  (pre-exchange part of attn^T), and
            # fold its contribution to out^T right away: oTpre = sum v@SR
            oTpre_ps = pB.tile([128, 512], f32, tag="bp", name=f"oTpre{s}")
            for c in range(NC1):
                sr = srp.tile([128, 512], f16, tag="sr", name=f"sr{s}_{c}")
                nc.vector.tensor_tensor(sr[:], expT[c][:], R_sb[:],
                                        mybir.AluOpType.mult)
                nc.vector.tensor_tensor(sr[:], sr[:], bb2T_sb[:, c],
                                        mybir.AluOpType.add)
                nc.tensor.matmul(oTpre_ps[:], v_t[s][:, c], sr[:],
                                 start=(c == 0), stop=(c == NC1 - 1))
            oTpre = srp.tile([128, 512], f32, tag="oTpre", name=f"oTpre{s}")
            nc.scalar.copy(oTpre[:], oTpre_ps[:])
            SR_t[s] = oTpre

        # ---- tail: out^T = oTpre + sum_c v_c @ recv_c, transpose, store -
        for s in range(BPC):
            oT_ps = pB.tile([128, 512], f32, tag="bp", name=f"oT{s}")
            for c in range(NC1):
                nc.tensor.matmul(oT_ps[:], v_t[s][:, c],
                                 RECV[:, :, s, c, :],
                                 start=(c == 0), stop=(c == NC1 - 1))

            oT_sb = outp.tile([128, L], f16, tag="oT", name=f"oTs{s}")
            nc.vector.tensor_tensor(oT_sb[:], oT_ps[:], SR_t[s][:],
                                    mybir.AluOpType.add)
            o_ps = pT.tile([128, 512], f16, tag="tp", name=f"ops{s}",
                           padded_shape=[128, 1024])
            for j in range(NC1):
                nc.tensor.transpose(o_ps[:, ts(j, 128)], oT_sb[:, ts(j, 128)],
                                    ident16)
            o_sb = outp.tile([128, NC1, DQ], f16, tag="o", name=f"os{s}")
            nc.scalar.copy(o_sb[:], o_ps[:].rearrange("p (j d) -> p j d",
                                                      j=NC1))
            nc.sync.dma_start(out_d[s].rearrange("(j p) d -> p j d", p=128),
                              o_sb[:])

        if DEBUG == "recv_end":
            nc.sync.dma_start(
                dbgR_d[:], RECV[:].rearrange("p m u b q -> p (m u b q)"))

    nc.compile()
    _cache["nc"] = nc
    return nc


def _l1perm(i):
    # rank-natural grouping via the AllToAll: no row permutation needed
    return np.arange(L)


def _prep_in_maps(query, key, value, sf, atten_mask, Wq, bq, Wk, bk, Wv, bv,
                  Wb1, bb1, Wb2, bb2):
    f16 = np.float16
    wqkv = np.ascontiguousarray(
        np.stack([np.asarray(Wq, f16), np.asarray(Wk, f16),
                  np.asarray(Wv, f16)], axis=1)
        .reshape(KT, 128, 3, DQ).transpose(1, 0, 2, 3))
    bias4 = np.ascontiguousarray(
        np.stack([np.asarray(bq, np.float32), np.asarray(bk, np.float32),
                  np.asarray(bv, np.float32), np.asarray(bb1, np.float32)],
                 axis=1))
    Wb1f = np.ascontiguousarray(
        np.asarray(Wb1, np.float32).reshape(2, 128, DMLP).transpose(1, 0, 2))
    Wb2_16 = np.asarray(Wb2, f16)
    bb2_mat = np.asarray(bb2, np.float32).reshape(L, L)
    sf32 = np.asarray(sf, np.float32)
    mask_u8 = np.asarray(atten_mask, np.uint8)

    def tr_in(x):
        # [4, l, din] -> [4, p(128), kt, l]
        xt = np.asarray(x, f16).transpose(0, 2, 1)
        return np.ascontiguousarray(
            xt.reshape(BPC, KT, 128, L).transpose(0, 2, 1, 3))

    in_maps = []
    for i in range(N_CORES):
        sl = slice(BPC * i, BPC * (i + 1))
        lp = _l1perm(i)
        sfp = sf32
        sfT = np.ascontiguousarray(
            sfp.T.reshape(2, 128, B).transpose(1, 0, 2))
        # mask^T with l1 permuted: [4, p=l2%128, c2, l1pos]
        mT = mask_u8[sl].transpose(0, 2, 1)[:, :, lp]
        mT = np.ascontiguousarray(
            mT.reshape(BPC, NC1, 128, L).transpose(0, 2, 1, 3))
        # bb2^T with l1 permuted: [p=l2%128, c2, l1pos]
        bb2T = np.ascontiguousarray(
            bb2_mat[lp].T.astype(f16).reshape(NC1, 128, L).transpose(1, 0, 2))
        in_maps.append({
            "qT": tr_in(query[sl][:, lp]),
            "kT": tr_in(key[sl]),
            "vT": tr_in(value[sl]),
            "mkT": mT,
            "sfT": sfT,
            "wqkv": wqkv,
            "bias4": bias4,
            "Wb1": Wb1f,
            "Wb2s": np.ascontiguousarray(Wb2_16[:, NSH * i: NSH * (i + 1)]),
            "bb2T": bb2T,
        })
    return in_maps


def kernel(**inputs) -> np.ndarray:
    from concourse import bass_utils
    nc = _build()
    in_maps = _prep_in_maps(**inputs)
    res = bass_utils.run_bass_kernel_spmd(
        nc, in_maps, core_ids=list(range(N_CORES)))
    out = np.empty((B, L, DQ), np.float32)
    for i, r in enumerate(res.results):
        out[BPC * i: BPC * (i + 1), _l1perm(i)] = \
            np.asarray(r["out"], np.float32)
    return out


# revision 47
# speedup vs baseline: 1.0118x; 1.0118x over previous
"""Trainium2 Bass kernel for nn_AttentionBiasHead.

Per-sample attention with a post-softmax additive bias produced by an MLP whose
output Linear is huge (128 x 262144).  Strategy (8 NeuronCores):

- Data-parallel over batch: core i owns samples [4i, 4i+4).
- The bias-MLP output Linear (Wb2) is column-sharded by l1: core i computes the
  bias rows l1 in [64i, 64(i+1)) for ALL 32 samples, TRANSPOSED to l2-major
  layout, then delivers each peer's 4-sample slice directly into the peer's
  SBUF with XOR-relative remote_dma (no DRAM bounce, no ncfw collective).
- Sample groups are XOR-permuted on the host (group g of core i = samples of
  core i^g) and the attention-side l1 row order is XOR-permuted the same way
  (row block m = rows of source i^m), so every remote send/receive slot is a
  compile-time-static AP in an SPMD program.
- The attention softmax runs transposed (scores^T with l2 on partitions): the
  denominator comes from a ones-vector matmul over partitions, the reciprocal
  is broadcast back with a rank-1 matmul, and the bias lands pre-transposed, so
  no tensor-engine transposes sit on the post-exchange critical path.
"""

import numpy as np

N_CORES = 8
B, L, DIN, DQ, DS, DMLP = 32, 512, 512, 128, 256, 128
BPC = B // N_CORES          # samples per core = 4
NSH = L * L // N_CORES      # bias-shard columns per core = 32768
KT = DIN // 128             # contraction tiles for projections = 4
NC1 = L // 128              # 128-row chunks per l dim = 4
NQ = 16                     # phase-B column groups (4 l1loc rows each)
SCALE = 1.0 / float(np.sqrt(DQ))
DEBUG = False

_cache = {}


def _build():
    if "nc" in _cache:
        return _cache["nc"]

    from contextlib import ExitStack

    import concourse.mybir as mybir
    import concourse.tile as tile
    from concourse import bacc
    from concourse.bass import ts, _add_dep_helper
    from concourse.masks import make_identity

    dt = mybir.dt
    f32, f16, u8 = dt.float32, dt.float16, dt.uint8

    nc = bacc.Bacc("TRN2", target_bir_lowering=False, debug=False,
                   num_devices=N_CORES, num_swdge_queues=4)

    # ---- per-core external tensors -------------------------------------
    qT_d = nc.dram_tensor("qT", [BPC, 128, KT, L], f16, kind="ExternalInput").ap()
    kT_d = nc.dram_tensor("kT", [BPC, 128, KT, L], f16, kind="ExternalInput").ap()
    vT_d = nc.dram_tensor("vT", [BPC, 128, KT, L], f16, kind="ExternalInput").ap()
    mkT_d = nc.dram_tensor("mkT", [BPC, 128, NC1, L], u8, kind="ExternalInput").ap()
    sfT_d = nc.dram_tensor("sfT", [128, DS // 128, B], f32, kind="ExternalInput").ap()
    wqkv_d = nc.dram_tensor("wqkv", [128, KT, 3, DQ], f16, kind="ExternalInput").ap()
    bias4_d = nc.dram_tensor("bias4", [128, 4], f32, kind="ExternalInput").ap()
    Wb1_d = nc.dram_tensor("Wb1", [128, DS // 128, DMLP], f32, kind="ExternalInput").ap()
    Wb2s_d = nc.dram_tensor("Wb2s", [DMLP, NSH], f16, kind="ExternalInput").ap()
    bb2T_d = nc.dram_tensor("bb2T", [128, NC1, L], f16, kind="ExternalInput").ap()
    out_d = nc.dram_tensor("out", [BPC, L, DQ], f16, kind="ExternalOutput").ap()
    if DEBUG:
        dbgB_d = nc.dram_tensor("dbgB", [128, 8192], f16, kind="ExternalOutput").ap()
        dbgR_d = nc.dram_tensor("dbgR", [128, 8192], f16, kind="ExternalOutput").ap()
        dbgE_d = nc.dram_tensor("dbgE", [128, 512], f16, kind="ExternalOutput").ap()
        dbgRec_d = nc.dram_tensor("dbgRec", [128, 512], f16, kind="ExternalOutput").ap()

    with tile.TileContext(nc) as tc, ExitStack() as ctx:
        consts = ctx.enter_context(tc.tile_pool(name="consts", bufs=1))

        # exchange buffers: BIG = my shard (transposed, grouped by dest),
        # RECV = assembled bias^T for my own 4 samples, written by peers
        BIG = consts.tile([128, N_CORES, BPC, NC1, 64], f16, name="BIG")
        RECV = consts.tile([128, N_CORES, BPC, NC1, 64], f16, name="RECV")

        # masks early on the sync queue (small, needed by phase C exp)
        mskp = ctx.enter_context(tc.tile_pool(name="mskp", bufs=BPC))
        mtile = {}
        for s in range(BPC):
            mtile[s] = mskp.tile([128, NC1, L], u8, tag="mt", name=f"mt{s}")
            nc.sync.dma_start(mtile[s][:], mkT_d[s])

        bb2T_sb = consts.tile([128, NC1, L], f16)
        nc.gpsimd.dma_start(bb2T_sb[:], bb2T_d[:])
        wqkv_sb = consts.tile([128, KT, 3, DQ], f16)
        nc.gpsimd.dma_start(wqkv_sb[:], wqkv_d[:])

        ident16 = consts.tile([128, 128], f16)
        make_identity(nc, ident16)
        c1 = consts.tile([128, 1], f16)
        nc.vector.memset(c1, 1.0)
        onesrow = consts.tile([1, 128], f16)
        nc.vector.memset(onesrow, 1.0)

        sfT_sb = consts.tile([128, DS // 128, B], f32)
        nc.gpsimd.dma_start(sfT_sb[:], sfT_d[:])
        Wb1_sb = consts.tile([128, DS // 128, DMLP], f32)
        nc.gpsimd.dma_start(Wb1_sb[:], Wb1_d[:])
        bias4_sb = consts.tile([128, 4], f32)
        nc.gpsimd.dma_start(bias4_sb[:], bias4_d[:])

        dram = ctx.enter_context(tc.tile_pool(name="dram", bufs=1, space="DRAM"))
        a2a_in = dram.tile([N_CORES, 128, 1024], f16, name="a2a_in")
        a2a_out = dram.tile([N_CORES, 128, 1024], f16, name="a2a_out")

        # ---- phase A: H^T = relu(Wb1^T @ sf^T + bb1)  [128, 32] --------
        with tc.tile_pool(name="htps", bufs=1, space="PSUM") as htps:
            ht_ps = htps.tile([128, 512], f32, name="ht_ps")
            for kt in range(DS // 128):
                nc.tensor.matmul(ht_ps[:, :B], Wb1_sb[:, kt], sfT_sb[:, kt],
                                 start=(kt == 0), stop=(kt == DS // 128 - 1))
            HT_sb = consts.tile([128, B], f16)
            nc.scalar.activation(HT_sb[:], ht_ps[:, :B],
                                 mybir.ActivationFunctionType.Relu,
                                 bias=bias4_sb[:, 3:4], scale=1.0)

        # ---- phase B: bias shard GEMM (4-stacked) + transpose to BIG ---
        pB = ctx.enter_context(tc.tile_pool(name="pB", bufs=2, space="PSUM"))
        pT = ctx.enter_context(tc.tile_pool(name="pT", bufs=2, space="PSUM"))
        stp = ctx.enter_context(tc.tile_pool(name="stp", bufs=3))

        NG = 8
        big_writes = []
        with tc.tile_pool(name="w2", bufs=4) as w2p:
            w2ts = []
            for g in range(64 // NG):
                w2t = w2p.tile([128, NG, 512], f16, tag="w2t", name=f"w2t{g}")
                # split the 8MB Wb2 stream across two DMA queues
                eng = nc.gpsimd if g % 2 == 0 else nc.scalar
                w2d = eng.dma_start(
                    w2t[:], Wb2s_d[:, ts(g, NG * 512)].rearrange(
                        "p (n w) -> p n w", w=512))
                w2ts.append(w2t)
                w2t_last = w2d
            for q in range(NQ):
                bp = pB.tile([128, 512], f32, tag="bp", name=f"bp{q}")
                for n in range(4):
                    t = 4 * q + n
                    nc.tensor.matmul(bp[ts(n, 32), :], HT_sb[:],
                                     w2ts[t // NG][:, t % NG],
                                     start=True, stop=True,
                                     tile_position=(0, 32 * n))
                st = stp.tile([128, 512], f16, tag="st", name=f"st{q}")
                eng = nc.scalar.copy if q % 2 == 0 else nc.vector.tensor_copy
                eng(st[:], bp[:])
                tp_ = pT.tile([128, 512], f16, tag="tp", name=f"tp{q}",
                              padded_shape=[128, 1024])
                for b in range(NC1):
                    nc.tensor.transpose(tp_[:, ts(b, 128)], st[:, ts(b, 128)],
                                        ident16)
                # scatter transposed block into BIG[(p), g, u, b, qn=4q+n]
                src = tp_[:].rearrange("p (b n g u) -> p b n g u",
                                       b=4, n=4, g=8, u=4)
                dst = BIG[:, :, :, :, ts(q, 4)].rearrange(
                    "p g u b n -> p b n g u")
                eng2 = nc.vector.tensor_copy if q % 2 == 0 else nc.scalar.copy
                big_writes.append(eng2(dst, src))

        # stage all 8 dest-shards to DRAM and run the AllToAll; ncfw does
        # the cross-core rendezvous internally (no in-kernel barrier)
        for g in range(N_CORES):
            nc.gpsimd.dma_start(
                a2a_in[g], BIG[:, g].rearrange("p u b q -> p (u b q)"))
        nc.gpsimd.collective_compute(
            "AllToAll", mybir.AluOpType.bypass,
            replica_groups=[list(range(N_CORES))],
            ins=[a2a_in.opt()], outs=[a2a_out.opt()],
        )
        for i in range(N_CORES):
            eng = (nc.sync, nc.scalar)[i % 2]
            eng.dma_start(
                RECV[:, i].rearrange("p u b q -> p (u b q)"), a2a_out[i])

        # ---- input loads: qk on sync (after Wb2), v on scalar gated ----
        inp = ctx.enter_context(tc.tile_pool(name="inp", bufs=BPC))
        qTin, kTin, vTin = {}, {}, {}
        for s in range(BPC):
            qTin[s] = inp.tile([128, KT, L], f16, tag="qTin", name=f"qTin{s}")
            qd = nc.sync.dma_start(qTin[s][:], qT_d[s])
            kTin[s] = inp.tile([128, KT, L], f16, tag="kTin", name=f"kTin{s}")
            kd = nc.sync.dma_start(kTin[s][:], kT_d[s])
            if s == 0:
                for dd in (qd, kd):
                    _add_dep_helper(dd.ins, w2t_last.ins, sync=True,
                                    reason="defer qk loads behind Wb2 stream")
        for s in range(BPC):
            vTin[s] = inp.tile([128, KT, L], f16, tag="vTin", name=f"vTin{s}")
            vd = nc.scalar.dma_start(vTin[s][:], vT_d[s])
            _add_dep_helper(vd.ins, w2t_last.ins, sync=True,
                            reason="defer v loads behind Wb2 stream")

        # ---- phase C PRE: everything independent of the exchange -------
        prj = ctx.enter_context(tc.tile_pool(name="prj", bufs=2))
        vpool = ctx.enter_context(tc.tile_pool(name="vpool", bufs=BPC))
        expp = ctx.enter_context(tc.tile_pool(name="expp", bufs=6))
        srp = ctx.enter_context(tc.tile_pool(name="srp", bufs=BPC * NC1))
        smal = ctx.enter_context(tc.tile_pool(name="smal", bufs=2 * BPC))
        rpool = ctx.enter_context(tc.tile_pool(name="rpool", bufs=BPC))
        pP = ctx.enter_context(tc.tile_pool(name="pP", bufs=2, space="PSUM"))
        pSC = ctx.enter_context(tc.tile_pool(name="pSC", bufs=2, space="PSUM"))
        atp = ctx.enter_context(tc.tile_pool(name="atp", bufs=NC1 + 2))
        outp = ctx.enter_context(tc.tile_pool(name="outp", bufs=3))
        SR_t, v_t = {}, {}

        for s in range(BPC):
            # projections (contraction over din on partitions)
            q_ps = pP.tile([128, 512], f32, tag="pp", name=f"qps{s}")
            for kt in range(KT):
                nc.tensor.matmul(q_ps[:], wqkv_sb[:, kt, 0], qTin[s][:, kt],
                                 start=(kt == 0), stop=(kt == KT - 1))
            qT_sb = prj.tile([128, L], f16, tag="qT", name=f"qT{s}")
            nc.scalar.activation(qT_sb[:], q_ps[:],
                                 mybir.ActivationFunctionType.Identity,
                                 bias=bias4_sb[:, 0:1], scale=1.0)

            k_ps = pP.tile([128, 512], f32, tag="pp", name=f"kps{s}")
            for kt in range(KT):
                nc.tensor.matmul(k_ps[:], wqkv_sb[:, kt, 1], kTin[s][:, kt],
                                 start=(kt == 0), stop=(kt == KT - 1))
            kT_sb = prj.tile([128, L], f16, tag="kT", name=f"kT{s}")
            nc.scalar.activation(kT_sb[:], k_ps[:],
                                 mybir.ActivationFunctionType.Identity,
                                 bias=bias4_sb[:, 1:2], scale=1.0)

            w_ps = pP.tile([128, 512], f32, tag="pp", name=f"wps{s}")
            for kt in range(KT):
                nc.tensor.matmul(w_ps[:], wqkv_sb[:, kt, 2], vTin[s][:, kt],
                                 start=(kt == 0), stop=(kt == KT - 1))
            vT_sb = prj.tile([128, L], f16, tag="vTs", name=f"vTs{s}")
            nc.vector.tensor_scalar_add(vT_sb[:], w_ps[:], bias4_sb[:, 2:3])
            v_ps = pT.tile([128, 512], f16, tag="tp", name=f"vps{s}",
                           padded_shape=[128, 1024])
            for j in range(NC1):
                nc.tensor.transpose(v_ps[:, ts(j, 128)], vT_sb[:, ts(j, 128)],
                                    ident16)
            v_sb = vpool.tile([128, NC1, DQ], f16, tag="v", name=f"v{s}")
            nc.vector.tensor_copy(v_sb[:], v_ps[:].rearrange(
                "p (j d) -> p j d", j=NC1))
            v_t[s] = v_sb

            # transposed scores + exp, per l2-chunk
            expT = {}
            for c in range(NC1):
                sc_ps = pSC.tile([128, 512], f32, tag="sp", name=f"sc{s}_{c}")
                nc.tensor.matmul(sc_ps[:], kT_sb[:, ts(c, 128)], qT_sb[:],
                                 start=True, stop=True)
                e = expp.tile([128, L], f16, tag="exp", name=f"exp{s}_{c}")
                nc.scalar.activation(e[:], sc_ps[:],
                                     mybir.ActivationFunctionType.Exp,
                                     bias=0.0, scale=SCALE)
                nc.vector.copy_predicated(e[:], mtile[s][:, c],
                                          c1[:].to_broadcast([128, 512]))
                expT[c] = e

            # denominator over partitions (all l2) via ones-matmul
            den = pSC.tile([1, 512], f32, tag="sp", name=f"den{s}",
                           padded_shape=[128, 512])
            for c in range(NC1):
                nc.tensor.matmul(den[:], c1[:], expT[c][:],
                                 start=(c == 0), stop=(c == NC1 - 1))
            rec32 = smal.tile([1, 512], f32, tag="r32", name=f"r32_{s}")
            nc.vector.reciprocal(rec32[:], den[:])
            rec16 = smal.tile([1, 512], f16, tag="r16", name=f"r16_{s}")
            nc.vector.tensor_copy(rec16[:], rec32[:])
            # broadcast reciprocal across partitions: ones[1,128]^T x rec16
            r_ps = pB.tile([128, 512], f32, tag="bp", name=f"rps{s}")
            nc.tensor.matmul(r_ps[:], onesrow[:], rec16[:],
                             start=True, stop=True)
            R_sb = rpool.tile([128, 512], f16, tag="R", name=f"R{s}")
            nc.scalar.copy(R_sb[:], r_ps[:])

            # SR = exp^T * R + bb2^T  (pre-exchange part of attn^T), and
            # fold its contribution to out^T right away: oTpre = sum v@SR
            oTpre_ps = pB.tile([128, 512], f32, tag="bp", name=f"oTpre{s}")
            for c in range(NC1):
                sr = srp.tile([128, 512], f16, tag="sr", name=f"sr{s}_{c}")
                nc.vector.tensor_tensor(sr[:], expT[c][:], R_sb[:],
                                        mybir.AluOpType.mult)
                nc.vector.tensor_tensor(sr[:], sr[:], bb2T_sb[:, c],
                                        mybir.AluOpType.add)
                nc.tensor.matmul(oTpre_ps[:], v_t[s][:, c], sr[:],
                                 start=(c == 0), stop=(c == NC1 - 1))
            oTpre = srp.tile([128, 512], f32, tag="oTpre", name=f"oTpre{s}")
            nc.scalar.copy(oTpre[:], oTpre_ps[:])
            SR_t[s] = oTpre

        # ---- tail: out^T = oTpre + sum_c v_c @ recv_c, transpose, store -
        for s in range(BPC):
            oT_ps = pB.tile([128, 512], f32, tag="bp", name=f"oT{s}")
            for c in range(NC1):
                nc.tensor.matmul(oT_ps[:], v_t[s][:, c],
                                 RECV[:, :, s, c, :],
                                 start=(c == 0), stop=(c == NC1 - 1))

            oT_sb = outp.tile([128, L], f16, tag="oT", name=f"oTs{s}")
            nc.vector.tensor_tensor(oT_sb[:], oT_ps[:], SR_t[s][:],
                                    mybir.AluOpType.add)
            o_ps = pT.tile([128, 512], f16, tag="tp", name=f"ops{s}",
                           padded_shape=[128, 1024])
            for j in range(NC1):
                nc.tensor.transpose(o_ps[:, ts(j, 128)], oT_sb[:, ts(j, 128)],
                                    ident16)
            o_sb = outp.tile([128, NC1, DQ], f16, tag="o", name=f"os{s}")
            nc.scalar.copy(o_sb[:], o_ps[:].rearrange("p (j d) -> p j d",
                                                      j=NC1))
            nc.sync.dma_start(out_d[s].rearrange("(j p) d -> p j d", p=128),
                              o_sb[:])

        if DEBUG == "recv_end":
            nc.sync.dma_start(
                dbgR_d[:], RECV[:].rearrange("p m u b q -> p (m u b q)"))

    nc.compile()
    _cache["nc"] = nc
    return nc


def _l1perm(i):
    # rank-natural grouping via the AllToAll: no row permutation needed
    return np.arange(L)


def _prep_in_maps(query, key, value, sf, atten_mask, Wq, bq, Wk, bk, Wv, bv,
                  Wb1, bb1, Wb2, bb2):
    f16 = np.float16
    wqkv = np.ascontiguousarray(
        np.stack([np.asarray(Wq, f16), np.asarray(Wk, f16),
                  np.asarray(Wv, f16)], axis=1)
        .reshape(KT, 128, 3, DQ).transpose(1, 0, 2, 3))
    bias4 = np.ascontiguousarray(
        np.stack([np.asarray(bq, np.float32), np.asarray(bk, np.float32),
                  np.asarray(bv, np.float32), np.asarray(bb1, np.float32)],
                 axis=1))
    Wb1f = np.ascontiguousarray(
        np.asarray(Wb1, np.float32).reshape(2, 128, DMLP).transpose(1, 0, 2))
    Wb2_16 = np.asarray(Wb2, f16)
    bb2_mat = np.asarray(bb2, np.float32).reshape(L, L)
    sf32 = np.asarray(sf, np.float32)
    mask_u8 = np.asarray(atten_mask, np.uint8)

    def tr_in(x):
        # [4, l, din] -> [4, p(128), kt, l]
        xt = np.asarray(x, f16).transpose(0, 2, 1)
        return np.ascontiguousarray(
            xt.reshape(BPC, KT, 128, L).transpose(0, 2, 1, 3))

    in_maps = []
    for i in range(N_CORES):
        sl = slice(BPC * i, BPC * (i + 1))
        lp = _l1perm(i)
        sfp = sf32
        sfT = np.ascontiguousarray(
            sfp.T.reshape(2, 128, B).transpose(1, 0, 2))
        # mask^T with l1 permuted: [4, p=l2%128, c2, l1pos]
        mT = mask_u8[sl].transpose(0, 2, 1)[:, :, lp]
        mT = np.ascontiguousarray(
            mT.reshape(BPC, NC1, 128, L).transpose(0, 2, 1, 3))
        # bb2^T with l1 permuted: [p=l2%128, c2, l1pos]
        bb2T = np.ascontiguousarray(
            bb2_mat[lp].T.astype(f16).reshape(NC1, 128, L).transpose(1, 0, 2))
        in_maps.append({
            "qT": tr_in(query[sl][:, lp]),
            "kT": tr_in(key[sl]),
            "vT": tr_in(value[sl]),
            "mkT": mT,
            "sfT": sfT,
            "wqkv": wqkv,
            "bias4": bias4,
            "Wb1": Wb1f,
            "Wb2s": np.ascontiguousarray(Wb2_16[:, NSH * i: NSH * (i + 1)]),
            "bb2T": bb2T,
        })
    return in_maps


def kernel(**inputs) -> np.ndarray:
    from concourse import bass_utils
    nc = _build()
    in_maps = _prep_in_maps(**inputs)
    res = bass_utils.run_bass_kernel_spmd(
        nc, in_maps, core_ids=list(range(N_CORES)))
    out = np.empty((B, L, DQ), np.float32)
    for i, r in enumerate(res.results):
        out[BPC * i: BPC * (i + 1), _l1perm(i)] = \
            np.asarray(r["out"], np.float32)
    return out
